# revision 17
# baseline (speedup 1.0000x reference)
"""Trainium2 Bass kernel for the decoupled sparse-attention layer.

Sharding: 8 cores = 2 batch x 4 GQA head-groups. Core i handles batch
b=i//4 and query heads [4g..4g+4) with KV head g, g=i%4. Each core
computes a partial output y_partial = attn_heads @ Wo_rows(group); the
host sums the 4 group partials per batch element.

Three-phase schedule to keep the PE array continuously busy (HAM):
  1. projections for all 8 t-chunks (bf16 GEMM stream) + RoPE +
     incremental KV pooling, q/k assembled via stream_shuffle RoPE
     (no SBUF->SBUF DMAs, all partition bases 32-aligned)
  2. V transposes for the P*V layout, kTd duplication, vsum
  3. attention per chunk with output-projection matmuls interleaved
     as PE fillers; exp on scalar engine; 1/sum via exp(-ln(sum)).

Per-core layouts (feature dim on partitions):
  xT      [2048, 4096] bf16 input activations (host pre-transposes)
  W_all   [2048, 384] bf16 fused projection weights, output cols:
            [0:128)   q_sem 4 heads x 32, scaled by exp(ls_h)/sqrt(32)
            [128:256) q_geo head-major [x1(16)|x2(16)] per head, scaled
            [256:288) k_sem 32
            [288:320) k_geo [x1(16)|x2(16)]
            [320:384) v 64
  q01_all/q23_all [128, T]: per 64-row head slot [sem 32|x1' 16|x2' 16]
  Keys: 1152 padded slots = [48 mem-blocks | 80 pad | 1024 local].
"""

import numpy as np

B, T, D = 2, 4096, 2048
H, HKV, DS, DG, DV = 16, 4, 32, 32, 64
MB, LW = 64, 1024
REMOTE = T - LW            # 3072
NBLK = REMOTE // MB        # 48
NKEY = NBLK + LW           # 1072
KPAD = 128 + LW            # 1152 padded key slots
NKT = KPAD // 128          # 9 key tiles
TC = 512                   # t-chunk size
NC_CHUNKS = T // TC        # 8
ROPE_BASE = 10000.0
MASK_BIAS = 80.0

_PROG = None

SWAP16 = list(range(16, 32)) + list(range(0, 16))


def _active_tiles(c):
    """Key tiles (tile_idx, nrows) visible to query chunk c, plus which
    tiles need the mask path."""
    tiles = [(0, 8 * (c + 1))] if c <= 5 else [(0, NBLK)]
    if c >= 6:
        nloc = (c - 5) * TC
        for t in range(1, 1 + nloc // 128):
            tiles.append((t, 128))
    masked = set()
    if c <= 5:
        masked.add(0)
    else:
        for t, n in tiles[1:]:
            maxpos = REMOTE + t * 128 - 1
            if maxpos > 512 * c:
                masked.add(t)
    return tiles, masked


def _build_program():
    from contextlib import ExitStack
    import concourse.bass as bass
    import concourse.bacc as bacc
    import concourse.tile as tile
    from concourse import mybir

    f32 = mybir.dt.float32
    f32r = mybir.dt.float32r
    bf16 = mybir.dt.bfloat16
    Alu = mybir.AluOpType
    Act = mybir.ActivationFunctionType

    nc = bacc.Bacc()
    xT = nc.declare_dram_parameter("xT", [D, T], bf16, isOutput=False)
    W_all = nc.declare_dram_parameter("W_all", [D, 384], bf16, isOutput=False)
    Wo = nc.declare_dram_parameter("Wo", [256, D], bf16, isOutput=False)
    c32d = nc.declare_dram_parameter("c32d", [32, T], f32, isOutput=False)
    s32d = nc.declare_dram_parameter("s32d", [32, T], f32, isOutput=False)
    kpos = nc.declare_dram_parameter("kpos", [KPAD], f32, isOutput=False)
    qpos = nc.declare_dram_parameter("qpos", [T], f32, isOutput=False)
    ident = nc.declare_dram_parameter("ident", [64, 64], f32, isOutput=False)
    y = nc.declare_dram_parameter("y", [T, D], bf16, isOutput=True)

    with tile.TileContext(nc) as tc, ExitStack() as ctx:
        persist = ctx.enter_context(tc.tile_pool(name="persist", bufs=1))
        xpool = ctx.enter_context(tc.tile_pool(name="x", bufs=2))
        tmp = ctx.enter_context(tc.tile_pool(name="tmp", bufs=2))
        epool = ctx.enter_context(tc.tile_pool(name="e", bufs=4))
        mpool = ctx.enter_context(tc.tile_pool(name="m", bufs=4))
        ypool = ctx.enter_context(tc.tile_pool(name="y", bufs=3))
        npool = ctx.enter_context(tc.tile_pool(name="n", bufs=2))

        # ---- persistent SBUF tensors ----
        wall_sb = persist.tile([128, 16, 384], bf16)
        nc.sync.dma_start(
            out=wall_sb,
            in_=bass.AP(tensor=W_all, offset=0,
                        ap=[[384, 128], [384 * 128, 16], [1, 384]]))
        wo_sb = persist.tile([128, 2, D], bf16)
        C128 = persist.tile([128, T], f32)
        S128 = persist.tile([128, T], f32)
        for qd in range(4):
            nc.scalar.dma_start(out=C128[32 * qd:32 * qd + 32, :], in_=c32d[:, :])
            nc.scalar.dma_start(out=S128[32 * qd:32 * qd + 32, :], in_=s32d[:, :])
        nc.scalar.dma_start(
            out=wo_sb,
            in_=bass.AP(tensor=Wo, offset=0,
                        ap=[[D, 128], [D * 128, 2], [1, D]]))
        ident_sb = persist.tile([64, 64], f32)
        nc.scalar.dma_start(out=ident_sb, in_=ident[:, :])
        kpos_sb = persist.tile([128, NKT], f32)
        nc.scalar.dma_start(
            out=kpos_sb,
            in_=bass.AP(tensor=kpos, offset=0, ap=[[1, 128], [128, NKT]]))

        q01_all = persist.tile([128, T], f32r)
        q23_all = persist.tile([128, T], f32r)
        aT01 = persist.tile([128, T], bf16)
        aT23 = persist.tile([128, T], bf16)
        kTd = persist.tile([128, KPAD], f32r)   # [sem32|x1'16|x2'16] dup'd
        vT = persist.tile([64, KPAD], f32)
        nc.vector.memset(vT, 0.0)
        v2 = persist.tile([128, NKT, 65], bf16)  # [key, dv | ones]
        onesrc = persist.tile([128, 1], f32)
        nc.vector.memset(onesrc, 1.0)
        nc.vector.tensor_copy(out=v2[0:NBLK, 0, 64:65], in_=onesrc[0:NBLK, :])
        for t in range(1, NKT):
            nc.vector.tensor_copy(out=v2[:, t, 64:65], in_=onesrc)
        negb = persist.tile([128, 1], f32)
        nc.vector.memset(negb, -MASK_BIAS)
        vsum = persist.tile([64, 1], f32)

        qsrc = [q01_all, q01_all, q23_all, q23_all]
        qb4 = [0, 64, 0, 64]          # 64-row slot base per head
        aTs = [aT01, aT01, aT23, aT23]

        # ================= PHASE 1: projections =================
        with tc.tile_pool(name="psp", bufs=2, space="PSUM") as ps_proj, \
             tc.tile_pool(name="pspv", bufs=2, space="PSUM") as ps_pv:
            for c in range(NC_CHUNKS):
                lo = c * TC
                sl = slice(lo, lo + TC)
                xt = xpool.tile([128, 16, TC], bf16, tag="xt")
                if c == 0:
                    for kk in range(16):
                        nc.sync.dma_start(
                            out=xt[:, kk, :],
                            in_=bass.AP(tensor=xT, offset=kk * 128 * T + lo,
                                        ap=[[T, 128], [1, TC]]))
                else:
                    nc.sync.dma_start(
                        out=xt,
                        in_=bass.AP(tensor=xT, offset=lo,
                                    ap=[[T, 128], [T * 128, 16], [1, TC]]))
                psA = ps_proj.tile([128, TC], f32, tag="psA")
                psB = ps_proj.tile([128, TC], f32, tag="psB")
                psC = ps_proj.tile([128, TC], f32, tag="psC")
                for kk in range(16):
                    st, sp = kk == 0, kk == 15
                    w = wall_sb[:, kk, :]
                    xk = xt[:, kk, :]
                    nc.tensor.matmul(out=psA, lhsT=w[:, 0:128], rhs=xk,
                                     start=st, stop=sp)
                    nc.tensor.matmul(out=psB, lhsT=w[:, 128:256], rhs=xk,
                                     start=st, stop=sp)
                    nc.tensor.matmul(out=psC, lhsT=w[:, 256:384], rhs=xk,
                                     start=st, stop=sp)

                # q_sem copies into the 64-row head slots
                nc.scalar.copy(out=q01_all[0:32, sl], in_=psA[0:32, :])
                nc.scalar.copy(out=q01_all[64:96, sl], in_=psA[32:64, :])
                nc.scalar.copy(out=q23_all[0:32, sl], in_=psA[64:96, :])
                nc.scalar.copy(out=q23_all[64:96, sl], in_=psA[96:128, :])
                # q_geo RoPE for all 4 heads: [128,512] mul/shuffle ops,
                # then per-head direct adds into the q tiles
                swq = tmp.tile([128, TC], f32, tag="swq")
                t1q = tmp.tile([128, TC], f32, tag="t1q")
                nc.vector.stream_shuffle(out=swq, in_=psB, mask=SWAP16)
                nc.vector.tensor_mul(t1q, psB, C128[:, sl])
                nc.vector.tensor_mul(swq, swq, S128[:, sl])
                for h in range(4):
                    nc.vector.tensor_add(
                        qsrc[h][qb4[h] + 32:qb4[h] + 64, sl],
                        t1q[32 * h:32 * h + 32, :], swq[32 * h:32 * h + 32, :])

                # k side: rope geo, then pool (c<=5) or copy local (c>=6)
                if c <= 5:
                    ktmp = tmp.tile([64, TC], f32, tag="ktmp")
                    nc.scalar.copy(out=ktmp[0:32, :], in_=psC[0:32, :])
                    swp = tmp.tile([64, TC], f32, tag="swp")
                    t1 = tmp.tile([32, TC], f32, tag="t1")
                    t2 = tmp.tile([32, TC], f32, tag="t2")
                    blk = psC[32:64, :]
                    nc.vector.stream_shuffle(out=swp[32:64, :], in_=blk, mask=SWAP16)
                    nc.vector.tensor_mul(t1, blk, C128[0:32, sl])
                    nc.vector.tensor_mul(t2, swp[32:64, :], S128[32:64, sl])
                    nc.vector.tensor_add(ktmp[32:64, :], t1, t2)
                    bs = slice(c * 8, (c + 1) * 8)
                    with nc.allow_low_precision(reason="fp32r pooled keys"):
                        nc.vector.tensor_reduce(
                            out=kTd[0:64, bs],
                            in_=ktmp.rearrange("p (n w) -> p n w", w=MB),
                            axis=mybir.AxisListType.X, op=Alu.add)
                    nc.vector.tensor_scalar_mul(kTd[0:64, bs], kTd[0:64, bs], 1.0 / MB)
                    nc.vector.tensor_reduce(
                        out=vT[:, bs],
                        in_=psC[64:128, :].rearrange("p (n w) -> p n w", w=MB),
                        axis=mybir.AxisListType.X, op=Alu.add)
                    nc.vector.tensor_scalar_mul(vT[:, bs], vT[:, bs], 1.0 / MB)
                else:
                    loff = 128 + (c - 6) * TC
                    lsl = slice(loff, loff + TC)
                    nc.scalar.copy(out=kTd[0:32, lsl], in_=psC[0:32, :])
                    swp = tmp.tile([64, TC], f32, tag="swp")
                    t1 = tmp.tile([32, TC], f32, tag="t1")
                    t2 = tmp.tile([32, TC], f32, tag="t2")
                    blk = psC[32:64, :]
                    nc.vector.stream_shuffle(out=swp[32:64, :], in_=blk, mask=SWAP16)
                    nc.vector.tensor_mul(t1, blk, C128[0:32, sl])
                    nc.vector.tensor_mul(t2, swp[32:64, :], S128[32:64, sl])
                    nc.vector.tensor_add(kTd[32:64, lsl], t1, t2)
                    nc.scalar.copy(out=vT[:, lsl], in_=psC[64:128, :])

            # ===== PHASE 2: kTd dup, V transposes, vsum =====
            nc.scalar.copy(out=kTd[64:128, :], in_=kTd[0:64, :])
            pv = ps_pv.tile([128, 64], f32, tag="pv")
            nc.tensor.transpose(out=pv[0:NBLK, :], in_=vT[:, 0:NBLK],
                                identity=ident_sb)
            nc.scalar.copy(out=v2[0:NBLK, 0, 0:64], in_=pv[0:NBLK, :])
            for i in range(8):
                pvl = ps_pv.tile([128, 64], f32, tag="pv")
                nc.tensor.transpose(out=pvl, in_=vT[:, 128 + 128 * i:256 + 128 * i],
                                    identity=ident_sb)
                nc.scalar.copy(out=v2[:, 1 + i, 0:64], in_=pvl)
            nc.vector.tensor_reduce(out=vsum, in_=vT, axis=mybir.AxisListType.X,
                                    op=Alu.add)
            nc.vector.tensor_scalar_mul(vsum, vsum, 1.0 / float(NKEY))

        # ================= PHASE 3: attention + out-proj =================
        with tc.tile_pool(name="pssc", bufs=2, space="PSUM") as ps_sc, \
             tc.tile_pool(name="psout", bufs=4, space="PSUM") as ps_out, \
             tc.tile_pool(name="psy", bufs=2, space="PSUM") as ps_y:

            def outproj_unit(tt, nn):
                tsl = slice(tt * 128, (tt + 1) * 128)
                nsl = slice(nn * 512, (nn + 1) * 512)
                yp = ps_y.tile([128, 512], f32, tag="yp")
                nc.tensor.matmul(out=yp, lhsT=aT01[:, tsl],
                                 rhs=wo_sb[:, 0, nsl], start=True, stop=False)
                nc.tensor.matmul(out=yp, lhsT=aT23[:, tsl],
                                 rhs=wo_sb[:, 1, nsl], start=False, stop=True)
                y_sb = ypool.tile([128, 512], bf16, tag="ysb")
                nc.vector.tensor_copy(out=y_sb, in_=yp)
                nc.sync.dma_start(out=y[tsl, nsl], in_=y_sb)

            fillers = []          # pending outproj units from chunk c-1

            for c in range(NC_CHUNKS):
                lo = c * TC
                sl = slice(lo, lo + TC)
                tiles, masked = _active_tiles(c)
                qpos_t = mpool.tile([128, TC], f32, tag="qp", bufs=2)
                nc.scalar.dma_start(
                    out=qpos_t,
                    in_=bass.AP(tensor=qpos, offset=lo, ap=[[0, 128], [1, TC]]))
                mdict = {}
                for (mt, n) in tiles:
                    if mt in masked:
                        m_sb = mpool.tile([128, TC], f32, tag="mask")
                        nc.vector.tensor_scalar(
                            out=m_sb[0:n, :], in0=qpos_t[0:n, :],
                            scalar1=kpos_sb[0:n, mt:mt + 1], scalar2=None,
                            op0=Alu.is_ge)
                        mdict[mt] = m_sb

                outp = [ps_out.tile([65, TC], f32, tag="out", name=f"outp{c}_{h}")
                        for h in range(4)]
                last_ti = len(tiles) - 1

                def score_exp(h, kt, n, ks):
                    qb = qb4[h]
                    sc = ps_sc.tile([128, TC], f32, tag="sc", name=f"sc{c}_{h}_{kt}")
                    nc.tensor.matmul(out=sc[0:n, :],
                                     lhsT=kTd[qb:qb + 64, ks],
                                     rhs=qsrc[h][qb:qb + 64, sl],
                                     start=True, stop=True)
                    e_sb = epool.tile([128, TC], bf16, tag="e",
                                      name=f"e{c}_{h}_{kt}")
                    if kt in mdict:
                        m_sb = mdict[kt]
                        nc.vector.scalar_tensor_tensor(
                            out=sc[0:n, :], in0=sc[0:n, :],
                            scalar=MASK_BIAS, in1=m_sb[0:n, :],
                            op0=Alu.add, op1=Alu.mult)
                        nc.scalar.activation(out=e_sb[0:n, :], in_=sc[0:n, :],
                                             func=Act.Exp, bias=negb[0:n, :])
                    else:
                        nc.scalar.activation(out=e_sb[0:n, :], in_=sc[0:n, :],
                                             func=Act.Exp)
                    return e_sb

                for ti, (kt, n) in enumerate(tiles):
                    ks = slice(kt * 128, kt * 128 + n)
                    if ti < last_ti:
                        for pair in ((0, 1), (2, 3)):
                            es = {h: score_exp(h, kt, n, ks) for h in pair}
                            for _ in range(2):
                                if fillers:
                                    outproj_unit(*fillers.pop(0))
                            for h in pair:
                                nc.tensor.matmul(out=outp[h],
                                                 lhsT=v2[0:n, kt, :],
                                                 rhs=es[h][0:n, :],
                                                 start=(ti == 0), stop=False)
                    else:
                        # last tile: all exps precede the stop-PVs so the
                        # Ln batch isn't interleaved with stray exps
                        es = {h: score_exp(h, kt, n, ks) for h in range(4)}
                        for _ in range(2):
                            if fillers:
                                outproj_unit(*fillers.pop(0))
                        for h in range(4):
                            nc.tensor.matmul(out=outp[h], lhsT=v2[0:n, kt, :],
                                             rhs=es[h][0:n, :],
                                             start=(ti == 0), stop=True)

                # free outp banks fast: unnormalized aT + denom row copies
                den4 = npool.tile([1, 4 * TC], f32, tag="den4", bufs=1,
                                  name=f"den4_{c}")
                for h in range(4):
                    nc.scalar.copy(out=den4[0:1, h * TC:(h + 1) * TC],
                                   in_=outp[h][64:65, :])
                    base = qb4[h]
                    nc.vector.tensor_copy(out=aTs[h][base:base + 64, sl],
                                          in_=outp[h][0:64, :])
                # batched 1/denominator via exp(-ln); bf16 recip
                nc.scalar.activation(out=den4, in_=den4, func=Act.Ln)
                er4 = npool.tile([1, 4 * TC], bf16, tag="er4", bufs=1,
                                 name=f"er4_{c}")
                nc.scalar.activation(out=er4, in_=den4, func=Act.Exp, scale=-1.0)
                # broadcast recips into per-pair [128,TC] tiles, normalize in place
                for ti2, dst in enumerate((aT01, aT23)):
                    rb = npool.tile([128, TC], bf16, tag="rb", bufs=2,
                                    name=f"rb_{c}_{ti2}")
                    rbt = npool.tile([64, TC], bf16, tag="rbt", bufs=2,
                                     name=f"rbt_{c}_{ti2}")
                    h0, h1 = (0, 1) if ti2 == 0 else (2, 3)
                    nc.gpsimd.partition_broadcast(
                        out_ap=rb[0:64, :], in_ap=er4[0:1, h0 * TC:(h0 + 1) * TC])
                    nc.gpsimd.partition_broadcast(
                        out_ap=rbt, in_ap=er4[0:1, h1 * TC:(h1 + 1) * TC])
                    nc.vector.tensor_copy(out=rb[64:128, :], in_=rbt)
                    nc.vector.tensor_mul(dst[:, sl], dst[:, sl], rb)

                if c == 0:
                    # uniform rows q in [0, 63): probs = 1/NKEY over all keys
                    for dst in (aT01, aT23):
                        for base in (0, 64):
                            nc.vector.tensor_copy(
                                out=dst[base:base + 64, 0:63],
                                in_=vsum.broadcast_to([64, 63]))

                # flush remaining fillers of chunk c-1, queue chunk c's units
                while fillers:
                    outproj_unit(*fillers.pop(0))
                fillers = [(c * 4 + tt, nn) for tt in range(4) for nn in range(4)]

            while fillers:
                outproj_unit(*fillers.pop(0))
    nc.finalize()
    return nc


def _host_inputs(x, Wq_sem, Wk_sem, Wq_geo, Wk_geo, Wv, Wo, logit_scale):
    """Build the 8 per-core input maps."""
    import ml_dtypes
    bf16 = ml_dtypes.bfloat16
    pos = np.arange(T, dtype=np.float64)
    inv = 1.0 / (ROPE_BASE ** (np.arange(0, DG, 2, dtype=np.float64) / DG))
    ang = pos[:, None] * inv[None, :]              # [T, 16]
    cosT = np.cos(ang).T.astype(np.float32)        # [16, T]
    sinT = np.sin(ang).T.astype(np.float32)
    c32 = np.concatenate([cosT, cosT], axis=0)     # [32, T]
    s32 = np.concatenate([-sinT, sinT], axis=0)
    kpos = np.full(KPAD, 1e9, dtype=np.float32)
    kpos[:NBLK] = np.arange(NBLK) * MB + (MB - 1)
    kpos[128:] = np.arange(REMOTE, T)
    qpos = np.arange(T, dtype=np.float32)
    ident = np.eye(64, dtype=np.float32)
    xTs = [np.ascontiguousarray(x[b].T).astype(bf16) for b in range(B)]

    scale = np.exp(logit_scale.astype(np.float64)).astype(np.float32)
    in_maps = []
    for core in range(8):
        b, g = core // 4, core % 4
        W = np.empty((D, 384), dtype=np.float32)
        for h in range(4):
            gh = 4 * g + h
            s = scale[gh] / np.sqrt(np.float32(DS))
            W[:, h * 32:(h + 1) * 32] = Wq_sem[:, gh * DS:(gh + 1) * DS] * s
            W[:, 128 + 32 * h:128 + 32 * h + 16] = \
                Wq_geo[:, gh * DG:gh * DG + 16] * s
            W[:, 128 + 32 * h + 16:128 + 32 * (h + 1)] = \
                Wq_geo[:, gh * DG + 16:(gh + 1) * DG] * s
        W[:, 256:288] = Wk_sem[:, g * DS:(g + 1) * DS]
        W[:, 288:304] = Wk_geo[:, g * DG:g * DG + 16]
        W[:, 304:320] = Wk_geo[:, g * DG + 16:(g + 1) * DG]
        W[:, 320:384] = Wv[:, g * DV:(g + 1) * DV]
        in_maps.append({
            "xT": xTs[b],
            "W_all": W.astype(bf16),
            "Wo": np.ascontiguousarray(Wo[g * 256:(g + 1) * 256, :]).astype(bf16),
            "c32d": c32, "s32d": s32, "kpos": kpos, "qpos": qpos,
            "ident": ident,
        })
    return in_maps


def kernel(x, Wq_sem, Wk_sem, Wq_geo, Wk_geo, Wv, Wo, logit_scale, _trace=False):
    global _PROG
    import sys
    if "/opt/trn_rl_repo" not in sys.path:
        sys.path.insert(0, "/opt/trn_rl_repo")
    from concourse.bass_utils import run_bass_kernel_spmd

    x = np.asarray(x, dtype=np.float32)
    in_maps = _host_inputs(np.asarray(x, np.float32),
                           np.asarray(Wq_sem, np.float32),
                           np.asarray(Wk_sem, np.float32),
                           np.asarray(Wq_geo, np.float32),
                           np.asarray(Wk_geo, np.float32),
                           np.asarray(Wv, np.float32),
                           np.asarray(Wo, np.float32),
                           np.asarray(logit_scale, np.float32))
    if _PROG is None:
        _PROG = _build_program()
    res = run_bass_kernel_spmd(_PROG, in_maps, list(range(8)), trace=_trace)
    outs = [res.results[i]["y"].astype(np.float32) for i in range(8)]
    out = np.empty((B, T, D), dtype=np.float32)
    for b in range(B):
        out[b] = outs[4 * b] + outs[4 * b + 1] + outs[4 * b + 2] + outs[4 * b + 3]
    if _trace:
        return out, res
    return out


# revision 18
# speedup vs baseline: 1.0266x; 1.0266x over previous
"""Trainium2 Bass kernel for the decoupled sparse-attention layer.

Sharding: 8 cores = 2 batch x 4 GQA head-groups. Core i handles batch
b=i//4 and query heads [4g..4g+4) with KV head g, g=i%4. Each core
computes a partial output y_partial = attn_heads @ Wo_rows(group); the
host sums the 4 group partials per batch element.

Three-phase schedule to keep the PE array continuously busy (HAM):
  1. projections for all 8 t-chunks (bf16 GEMM stream) + RoPE +
     incremental KV pooling, q/k assembled via stream_shuffle RoPE
     (no SBUF->SBUF DMAs, all partition bases 32-aligned)
  2. V transposes for the P*V layout, kTd duplication, vsum
  3. attention per chunk with output-projection matmuls interleaved
     as PE fillers; exp on scalar engine; 1/sum via exp(-ln(sum)).

Per-core layouts (feature dim on partitions):
  xT      [2048, 4096] bf16 input activations (host pre-transposes)
  W_all   [2048, 384] bf16 fused projection weights, output cols:
            [0:128)   q_sem 4 heads x 32, scaled by exp(ls_h)/sqrt(32)
            [128:256) q_geo head-major [x1(16)|x2(16)] per head, scaled
            [256:288) k_sem 32
            [288:320) k_geo [x1(16)|x2(16)]
            [320:384) v 64
  q01_all/q23_all [128, T]: per 64-row head slot [sem 32|x1' 16|x2' 16]
  Keys: 1152 padded slots = [48 mem-blocks | 80 pad | 1024 local].
"""

import numpy as np

B, T, D = 2, 4096, 2048
H, HKV, DS, DG, DV = 16, 4, 32, 32, 64
MB, LW = 64, 1024
REMOTE = T - LW            # 3072
NBLK = REMOTE // MB        # 48
NKEY = NBLK + LW           # 1072
KPAD = 128 + LW            # 1152 padded key slots
NKT = KPAD // 128          # 9 key tiles
TC = 512                   # t-chunk size
NC_CHUNKS = T // TC        # 8
ROPE_BASE = 10000.0
MASK_BIAS = 80.0

_PROG = None

SWAP16 = list(range(16, 32)) + list(range(0, 16))


def _active_tiles(c):
    """Key tiles (tile_idx, nrows) visible to query chunk c, plus which
    tiles need the mask path."""
    tiles = [(0, 8 * (c + 1))] if c <= 5 else [(0, NBLK)]
    if c >= 6:
        nloc = (c - 5) * TC
        for t in range(1, 1 + nloc // 128):
            tiles.append((t, 128))
    masked = set()
    if c <= 5:
        masked.add(0)
    else:
        for t, n in tiles[1:]:
            maxpos = REMOTE + t * 128 - 1
            if maxpos > 512 * c:
                masked.add(t)
    return tiles, masked


def _build_program():
    from contextlib import ExitStack
    import concourse.bass as bass
    import concourse.bacc as bacc
    import concourse.tile as tile
    from concourse import mybir

    f32 = mybir.dt.float32
    f32r = mybir.dt.float32r
    bf16 = mybir.dt.bfloat16
    Alu = mybir.AluOpType
    Act = mybir.ActivationFunctionType

    nc = bacc.Bacc()
    xT = nc.declare_dram_parameter("xT", [D, T], bf16, isOutput=False)
    W_all = nc.declare_dram_parameter("W_all", [D, 384], bf16, isOutput=False)
    Wo = nc.declare_dram_parameter("Wo", [256, D], bf16, isOutput=False)
    c32d = nc.declare_dram_parameter("c32d", [32, T], f32, isOutput=False)
    s32d = nc.declare_dram_parameter("s32d", [32, T], f32, isOutput=False)
    kpos = nc.declare_dram_parameter("kpos", [KPAD], f32, isOutput=False)
    qpos = nc.declare_dram_parameter("qpos", [T], f32, isOutput=False)
    ident = nc.declare_dram_parameter("ident", [64, 64], f32, isOutput=False)
    y = nc.declare_dram_parameter("y", [T, D], bf16, isOutput=True)

    with tile.TileContext(nc) as tc, ExitStack() as ctx:
        persist = ctx.enter_context(tc.tile_pool(name="persist", bufs=1))
        xpool = ctx.enter_context(tc.tile_pool(name="x", bufs=2))
        tmp = ctx.enter_context(tc.tile_pool(name="tmp", bufs=2))
        epool = ctx.enter_context(tc.tile_pool(name="e", bufs=4))
        mpool = ctx.enter_context(tc.tile_pool(name="m", bufs=4))
        ypool = ctx.enter_context(tc.tile_pool(name="y", bufs=3))
        npool = ctx.enter_context(tc.tile_pool(name="n", bufs=2))

        # ---- persistent SBUF tensors ----
        wall_sb = persist.tile([128, 16, 384], bf16)
        for wh in range(2):
            nc.sync.dma_start(
                out=wall_sb[:, 8 * wh:8 * wh + 8, :],
                in_=bass.AP(tensor=W_all, offset=wh * 8 * 128 * 384,
                            ap=[[384, 128], [384 * 128, 8], [1, 384]]))
        wo_sb = persist.tile([128, 2, D], bf16)
        C128 = persist.tile([128, T], f32)
        S128 = persist.tile([128, T], f32)
        for qd in range(4):
            nc.scalar.dma_start(out=C128[32 * qd:32 * qd + 32, :], in_=c32d[:, :])
            nc.scalar.dma_start(out=S128[32 * qd:32 * qd + 32, :], in_=s32d[:, :])
        nc.scalar.dma_start(
            out=wo_sb,
            in_=bass.AP(tensor=Wo, offset=0,
                        ap=[[D, 128], [D * 128, 2], [1, D]]))
        ident_sb = persist.tile([64, 64], f32)
        nc.scalar.dma_start(out=ident_sb, in_=ident[:, :])
        kpos_sb = persist.tile([128, NKT], f32)
        nc.scalar.dma_start(
            out=kpos_sb,
            in_=bass.AP(tensor=kpos, offset=0, ap=[[1, 128], [128, NKT]]))

        q01_all = persist.tile([128, T], f32r)
        q23_all = persist.tile([128, T], f32r)
        aT01 = persist.tile([128, T], bf16)
        aT23 = persist.tile([128, T], bf16)
        kTd = persist.tile([128, KPAD], f32r)   # [sem32|x1'16|x2'16] dup'd
        vT = persist.tile([64, KPAD], f32)
        nc.vector.memset(vT, 0.0)
        v2 = persist.tile([128, NKT, 65], bf16)  # [key, dv | ones]
        onesrc = persist.tile([128, 1], f32)
        nc.vector.memset(onesrc, 1.0)
        nc.vector.tensor_copy(out=v2[0:NBLK, 0, 64:65], in_=onesrc[0:NBLK, :])
        for t in range(1, NKT):
            nc.vector.tensor_copy(out=v2[:, t, 64:65], in_=onesrc)
        negb = persist.tile([128, 1], f32)
        nc.vector.memset(negb, -MASK_BIAS)
        vsum = persist.tile([64, 1], f32)

        qsrc = [q01_all, q01_all, q23_all, q23_all]
        qb4 = [0, 64, 0, 64]          # 64-row slot base per head
        aTs = [aT01, aT01, aT23, aT23]

        # ================= PHASE 1: projections =================
        with tc.tile_pool(name="psp", bufs=2, space="PSUM") as ps_proj, \
             tc.tile_pool(name="pspv", bufs=2, space="PSUM") as ps_pv:
            for c in range(NC_CHUNKS):
                lo = c * TC
                sl = slice(lo, lo + TC)
                xt = xpool.tile([128, 16, TC], bf16, tag="xt")
                if c == 0:
                    for kk in range(16):
                        nc.sync.dma_start(
                            out=xt[:, kk, :],
                            in_=bass.AP(tensor=xT, offset=kk * 128 * T + lo,
                                        ap=[[T, 128], [1, TC]]))
                else:
                    for xh in range(2):
                        nc.sync.dma_start(
                            out=xt[:, 8 * xh:8 * xh + 8, :],
                            in_=bass.AP(tensor=xT,
                                        offset=xh * 8 * 128 * T + lo,
                                        ap=[[T, 128], [T * 128, 8], [1, TC]]))
                psA = ps_proj.tile([128, TC], f32, tag="psA")
                psB = ps_proj.tile([128, TC], f32, tag="psB")
                psC = ps_proj.tile([128, TC], f32, tag="psC")
                for kk in range(16):
                    st, sp = kk == 0, kk == 15
                    w = wall_sb[:, kk, :]
                    xk = xt[:, kk, :]
                    nc.tensor.matmul(out=psA, lhsT=w[:, 0:128], rhs=xk,
                                     start=st, stop=sp)
                    nc.tensor.matmul(out=psB, lhsT=w[:, 128:256], rhs=xk,
                                     start=st, stop=sp)
                    nc.tensor.matmul(out=psC, lhsT=w[:, 256:384], rhs=xk,
                                     start=st, stop=sp)

                # q_sem copies into the 64-row head slots
                nc.scalar.copy(out=q01_all[0:32, sl], in_=psA[0:32, :])
                nc.scalar.copy(out=q01_all[64:96, sl], in_=psA[32:64, :])
                nc.scalar.copy(out=q23_all[0:32, sl], in_=psA[64:96, :])
                nc.scalar.copy(out=q23_all[64:96, sl], in_=psA[96:128, :])
                # q_geo RoPE for all 4 heads: [128,512] mul/shuffle ops,
                # then per-head direct adds into the q tiles
                swq = tmp.tile([128, TC], f32, tag="swq")
                t1q = tmp.tile([128, TC], f32, tag="t1q")
                nc.vector.stream_shuffle(out=swq, in_=psB, mask=SWAP16)
                nc.vector.tensor_mul(t1q, psB, C128[:, sl])
                nc.vector.tensor_mul(swq, swq, S128[:, sl])
                for h in range(4):
                    nc.vector.tensor_add(
                        qsrc[h][qb4[h] + 32:qb4[h] + 64, sl],
                        t1q[32 * h:32 * h + 32, :], swq[32 * h:32 * h + 32, :])

                # k side: rope geo, then pool (c<=5) or copy local (c>=6)
                if c <= 5:
                    ktmp = tmp.tile([64, TC], f32, tag="ktmp")
                    nc.scalar.copy(out=ktmp[0:32, :], in_=psC[0:32, :])
                    swp = tmp.tile([64, TC], f32, tag="swp")
                    t1 = tmp.tile([32, TC], f32, tag="t1")
                    t2 = tmp.tile([32, TC], f32, tag="t2")
                    blk = psC[32:64, :]
                    nc.vector.stream_shuffle(out=swp[32:64, :], in_=blk, mask=SWAP16)
                    nc.vector.tensor_mul(t1, blk, C128[0:32, sl])
                    nc.vector.tensor_mul(t2, swp[32:64, :], S128[32:64, sl])
                    nc.vector.tensor_add(ktmp[32:64, :], t1, t2)
                    bs = slice(c * 8, (c + 1) * 8)
                    with nc.allow_low_precision(reason="fp32r pooled keys"):
                        nc.vector.tensor_reduce(
                            out=kTd[0:64, bs],
                            in_=ktmp.rearrange("p (n w) -> p n w", w=MB),
                            axis=mybir.AxisListType.X, op=Alu.add)
                    nc.vector.tensor_scalar_mul(kTd[0:64, bs], kTd[0:64, bs], 1.0 / MB)
                    nc.vector.tensor_reduce(
                        out=vT[:, bs],
                        in_=psC[64:128, :].rearrange("p (n w) -> p n w", w=MB),
                        axis=mybir.AxisListType.X, op=Alu.add)
                    nc.vector.tensor_scalar_mul(vT[:, bs], vT[:, bs], 1.0 / MB)
                else:
                    loff = 128 + (c - 6) * TC
                    lsl = slice(loff, loff + TC)
                    nc.scalar.copy(out=kTd[0:32, lsl], in_=psC[0:32, :])
                    swp = tmp.tile([64, TC], f32, tag="swp")
                    t1 = tmp.tile([32, TC], f32, tag="t1")
                    t2 = tmp.tile([32, TC], f32, tag="t2")
                    blk = psC[32:64, :]
                    nc.vector.stream_shuffle(out=swp[32:64, :], in_=blk, mask=SWAP16)
                    nc.vector.tensor_mul(t1, blk, C128[0:32, sl])
                    nc.vector.tensor_mul(t2, swp[32:64, :], S128[32:64, sl])
                    nc.vector.tensor_add(kTd[32:64, lsl], t1, t2)
                    nc.scalar.copy(out=vT[:, lsl], in_=psC[64:128, :])

            # ===== PHASE 2: kTd dup, V transposes, vsum =====
            nc.scalar.copy(out=kTd[64:128, :], in_=kTd[0:64, :])
            pv = ps_pv.tile([128, 64], f32, tag="pv")
            nc.tensor.transpose(out=pv[0:NBLK, :], in_=vT[:, 0:NBLK],
                                identity=ident_sb)
            nc.scalar.copy(out=v2[0:NBLK, 0, 0:64], in_=pv[0:NBLK, :])
            for i in range(8):
                pvl = ps_pv.tile([128, 64], f32, tag="pv")
                nc.tensor.transpose(out=pvl, in_=vT[:, 128 + 128 * i:256 + 128 * i],
                                    identity=ident_sb)
                nc.scalar.copy(out=v2[:, 1 + i, 0:64], in_=pvl)
            nc.vector.tensor_reduce(out=vsum, in_=vT, axis=mybir.AxisListType.X,
                                    op=Alu.add)
            nc.vector.tensor_scalar_mul(vsum, vsum, 1.0 / float(NKEY))

        # ================= PHASE 3: attention + out-proj =================
        with tc.tile_pool(name="pssc", bufs=2, space="PSUM") as ps_sc, \
             tc.tile_pool(name="psout", bufs=4, space="PSUM") as ps_out, \
             tc.tile_pool(name="psy", bufs=2, space="PSUM") as ps_y:

            def outproj_unit(tt, nn, alt=False):
                tsl = slice(tt * 128, (tt + 1) * 128)
                nsl = slice(nn * 512, (nn + 1) * 512)
                yp = ps_y.tile([128, 512], f32, tag="yp")
                nc.tensor.matmul(out=yp, lhsT=aT01[:, tsl],
                                 rhs=wo_sb[:, 0, nsl], start=True, stop=False)
                nc.tensor.matmul(out=yp, lhsT=aT23[:, tsl],
                                 rhs=wo_sb[:, 1, nsl], start=False, stop=True)
                y_sb = ypool.tile([128, 512], bf16, tag="ysb")
                if alt and (tt + nn) % 2 == 1:
                    nc.scalar.copy(out=y_sb, in_=yp)
                else:
                    nc.vector.tensor_copy(out=y_sb, in_=yp)
                nc.sync.dma_start(out=y[tsl, nsl], in_=y_sb)

            fillers = []          # pending outproj units from chunk c-1

            for c in range(NC_CHUNKS):
                lo = c * TC
                sl = slice(lo, lo + TC)
                tiles, masked = _active_tiles(c)
                qpos_t = mpool.tile([128, TC], f32, tag="qp", bufs=2)
                nc.scalar.dma_start(
                    out=qpos_t,
                    in_=bass.AP(tensor=qpos, offset=lo, ap=[[0, 128], [1, TC]]))
                mdict = {}
                for (mt, n) in tiles:
                    if mt in masked:
                        m_sb = mpool.tile([128, TC], f32, tag="mask")
                        nc.vector.tensor_scalar(
                            out=m_sb[0:n, :], in0=qpos_t[0:n, :],
                            scalar1=kpos_sb[0:n, mt:mt + 1], scalar2=None,
                            op0=Alu.is_ge)
                        mdict[mt] = m_sb

                outp = [ps_out.tile([65, TC], f32, tag="out", name=f"outp{c}_{h}")
                        for h in range(4)]
                last_ti = len(tiles) - 1

                def score_exp(h, kt, n, ks):
                    qb = qb4[h]
                    sc = ps_sc.tile([128, TC], f32, tag="sc", name=f"sc{c}_{h}_{kt}")
                    nc.tensor.matmul(out=sc[0:n, :],
                                     lhsT=kTd[qb:qb + 64, ks],
                                     rhs=qsrc[h][qb:qb + 64, sl],
                                     start=True, stop=True)
                    e_sb = epool.tile([128, TC], bf16, tag="e",
                                      name=f"e{c}_{h}_{kt}")
                    if kt in mdict:
                        m_sb = mdict[kt]
                        nc.vector.scalar_tensor_tensor(
                            out=sc[0:n, :], in0=sc[0:n, :],
                            scalar=MASK_BIAS, in1=m_sb[0:n, :],
                            op0=Alu.add, op1=Alu.mult)
                        nc.scalar.activation(out=e_sb[0:n, :], in_=sc[0:n, :],
                                             func=Act.Exp, bias=negb[0:n, :])
                    else:
                        nc.scalar.activation(out=e_sb[0:n, :], in_=sc[0:n, :],
                                             func=Act.Exp)
                    return e_sb

                for ti, (kt, n) in enumerate(tiles):
                    ks = slice(kt * 128, kt * 128 + n)
                    if ti < last_ti:
                        for pair in ((0, 1), (2, 3)):
                            es = {h: score_exp(h, kt, n, ks) for h in pair}
                            for _ in range(2):
                                if fillers:
                                    outproj_unit(*fillers.pop(0))
                            for h in pair:
                                nc.tensor.matmul(out=outp[h],
                                                 lhsT=v2[0:n, kt, :],
                                                 rhs=es[h][0:n, :],
                                                 start=(ti == 0), stop=False)
                    else:
                        # last tile: all exps precede the stop-PVs so the
                        # Ln batch isn't interleaved with stray exps
                        es = {h: score_exp(h, kt, n, ks) for h in range(4)}
                        for _ in range(2):
                            if fillers:
                                outproj_unit(*fillers.pop(0))
                        for h in range(4):
                            nc.tensor.matmul(out=outp[h], lhsT=v2[0:n, kt, :],
                                             rhs=es[h][0:n, :],
                                             start=(ti == 0), stop=True)

                # free outp banks fast: unnormalized aT + denom row copies
                den4 = npool.tile([1, 4 * TC], f32, tag="den4", bufs=1,
                                  name=f"den4_{c}")
                for h in range(4):
                    nc.scalar.copy(out=den4[0:1, h * TC:(h + 1) * TC],
                                   in_=outp[h][64:65, :])
                    base = qb4[h]
                    nc.vector.tensor_copy(out=aTs[h][base:base + 64, sl],
                                          in_=outp[h][0:64, :])
                # batched 1/denominator via exp(-ln); bf16 recip
                nc.scalar.activation(out=den4, in_=den4, func=Act.Ln)
                er4 = npool.tile([1, 4 * TC], bf16, tag="er4", bufs=1,
                                 name=f"er4_{c}")
                nc.scalar.activation(out=er4, in_=den4, func=Act.Exp, scale=-1.0)
                # broadcast recips into per-pair [128,TC] tiles, normalize in place
                for ti2, dst in enumerate((aT01, aT23)):
                    rb = npool.tile([128, TC], bf16, tag="rb", bufs=2,
                                    name=f"rb_{c}_{ti2}")
                    rbt = npool.tile([64, TC], bf16, tag="rbt", bufs=2,
                                     name=f"rbt_{c}_{ti2}")
                    h0, h1 = (0, 1) if ti2 == 0 else (2, 3)
                    nc.gpsimd.partition_broadcast(
                        out_ap=rb[0:64, :], in_ap=er4[0:1, h0 * TC:(h0 + 1) * TC])
                    nc.gpsimd.partition_broadcast(
                        out_ap=rbt, in_ap=er4[0:1, h1 * TC:(h1 + 1) * TC])
                    nc.vector.tensor_copy(out=rb[64:128, :], in_=rbt)
                    nc.vector.tensor_mul(dst[:, sl], dst[:, sl], rb)

                if c == 0:
                    # uniform rows q in [0, 63): probs = 1/NKEY over all keys
                    for dst in (aT01, aT23):
                        for base in (0, 64):
                            nc.vector.tensor_copy(
                                out=dst[base:base + 64, 0:63],
                                in_=vsum.broadcast_to([64, 63]))

                # flush remaining fillers of chunk c-1, queue chunk c's units
                while fillers:
                    outproj_unit(*fillers.pop(0))
                fillers = [(c * 4 + tt, nn) for tt in range(4) for nn in range(4)]

            while fillers:
                outproj_unit(*fillers.pop(0), alt=True)
    nc.finalize()
    return nc


def _host_inputs(x, Wq_sem, Wk_sem, Wq_geo, Wk_geo, Wv, Wo, logit_scale):
    """Build the 8 per-core input maps."""
    import ml_dtypes
    bf16 = ml_dtypes.bfloat16
    pos = np.arange(T, dtype=np.float64)
    inv = 1.0 / (ROPE_BASE ** (np.arange(0, DG, 2, dtype=np.float64) / DG))
    ang = pos[:, None] * inv[None, :]              # [T, 16]
    cosT = np.cos(ang).T.astype(np.float32)        # [16, T]
    sinT = np.sin(ang).T.astype(np.float32)
    c32 = np.concatenate([cosT, cosT], axis=0)     # [32, T]
    s32 = np.concatenate([-sinT, sinT], axis=0)
    kpos = np.full(KPAD, 1e9, dtype=np.float32)
    kpos[:NBLK] = np.arange(NBLK) * MB + (MB - 1)
    kpos[128:] = np.arange(REMOTE, T)
    qpos = np.arange(T, dtype=np.float32)
    ident = np.eye(64, dtype=np.float32)
    xTs = [np.ascontiguousarray(x[b].T).astype(bf16) for b in range(B)]

    scale = np.exp(logit_scale.astype(np.float64)).astype(np.float32)
    in_maps = []
    for core in range(8):
        b, g = core // 4, core % 4
        W = np.empty((D, 384), dtype=np.float32)
        for h in range(4):
            gh = 4 * g + h
            s = scale[gh] / np.sqrt(np.float32(DS))
            W[:, h * 32:(h + 1) * 32] = Wq_sem[:, gh * DS:(gh + 1) * DS] * s
            W[:, 128 + 32 * h:128 + 32 * h + 16] = \
                Wq_geo[:, gh * DG:gh * DG + 16] * s
            W[:, 128 + 32 * h + 16:128 + 32 * (h + 1)] = \
                Wq_geo[:, gh * DG + 16:(gh + 1) * DG] * s
        W[:, 256:288] = Wk_sem[:, g * DS:(g + 1) * DS]
        W[:, 288:304] = Wk_geo[:, g * DG:g * DG + 16]
        W[:, 304:320] = Wk_geo[:, g * DG + 16:(g + 1) * DG]
        W[:, 320:384] = Wv[:, g * DV:(g + 1) * DV]
        in_maps.append({
            "xT": xTs[b],
            "W_all": W.astype(bf16),
            "Wo": np.ascontiguousarray(Wo[g * 256:(g + 1) * 256, :]).astype(bf16),
            "c32d": c32, "s32d": s32, "kpos": kpos, "qpos": qpos,
            "ident": ident,
        })
    return in_maps


def kernel(x, Wq_sem, Wk_sem, Wq_geo, Wk_geo, Wv, Wo, logit_scale, _trace=False):
    global _PROG
    import sys
    if "/opt/trn_rl_repo" not in sys.path:
        sys.path.insert(0, "/opt/trn_rl_repo")
    from concourse.bass_utils import run_bass_kernel_spmd

    x = np.asarray(x, dtype=np.float32)
    in_maps = _host_inputs(np.asarray(x, np.float32),
                           np.asarray(Wq_sem, np.float32),
                           np.asarray(Wk_sem, np.float32),
                           np.asarray(Wq_geo, np.float32),
                           np.asarray(Wk_geo, np.float32),
                           np.asarray(Wv, np.float32),
                           np.asarray(Wo, np.float32),
                           np.asarray(logit_scale, np.float32))
    if _PROG is None:
        _PROG = _build_program()
    res = run_bass_kernel_spmd(_PROG, in_maps, list(range(8)), trace=_trace)
    outs = [res.results[i]["y"].astype(np.float32) for i in range(8)]
    out = np.empty((B, T, D), dtype=np.float32)
    for b in range(B):
        out[b] = outs[4 * b] + outs[4 * b + 1] + outs[4 * b + 2] + outs[4 * b + 3]
    if _trace:
        return out, res
    return out
